# revision 21
# baseline (speedup 1.0000x reference)
"""Trainium2 Bass kernel for a dense transformer block (B=8, N=1024, C=1024,
H=16, D=64, HID=4096) with padding-masked attention.

Sharding: data-parallel over batch - one batch element per NeuronCore.

v2 design (vs v1 baseline):
- All matmul operands bf16 (weights host-converted; activations converted on
  the fly by the producing engine).  Halves DMA + SBUF, keeps 1 cyc/row PE.
- LayerNorm normalize folded into the following matmul:  LN(x)@Wg =
  (x*rstd)@(W*g) + rank-1 correction (-mu*rstd outer sum(W*g)) + W@b.  The
  rank-1 + bias terms ride the PSUM accumulation as one extra K<=2 matmul.
  Kills the per-tile sub/scale/bias elementwise chain of v1.
- Softmax denominator reciprocal broadcast via gpsimd partition_broadcast
  (SBUF only) instead of a DRAM DMA round-trip on the ACT queue.
- Scores processed in [128,512] PSUM chunks; exp on ACT; QKV q/k production
  matmuls interleaved into the attention stream so PE never idles while ACT
  exps run.
- proj + residual + LN2 stats interleaved; fc2 accumulates o-major in PSUM
  (no SBUF accumulation adds); x / x2 stay in SBUF (no DRAM spill).
- Engine discipline: SP queue = weight/x prefetch only; Pool = squares +
  partition broadcasts; DVE = everything touching PSUM; ACT = exp/gelu/sqrt.
"""

import os
import sys

for _p in ("/opt/trn_rl_repo",):
    if _p not in sys.path:
        sys.path.insert(0, _p)
os.environ.setdefault("MYCRO_LOCAL_CACHE", "1")

import numpy as np  # noqa: E402
import ml_dtypes  # noqa: E402

import concourse.bacc as bacc  # noqa: E402
import concourse.tile as tile  # noqa: E402
from concourse import mybir  # noqa: E402
from concourse.bass_utils import run_bass_kernel_spmd  # noqa: E402

f32 = mybir.dt.float32
bf16 = mybir.dt.bfloat16
AF = mybir.ActivationFunctionType
ALU = mybir.AluOpType
BF = ml_dtypes.bfloat16

B, N, C = 8, 1024, 1024
H, D = 16, 64
HID = 4 * C
CT = C // 128          # 8 c-tiles
NT = N // 128          # 8 n/k-tiles
HT = HID // 128        # 32 hid-tiles
SCALE = D ** -0.5
EPS = 1e-5
MASK_NEG = -10000.0

NCORES = 8


def build_program(repeat=1, stop_after=None):
    nc = bacc.Bacc("TRN2", target_bir_lowering=False, debug=False)

    xT = nc.dram_tensor("xT", [C, N], bf16, kind="ExternalInput").ap()
    maskv = nc.dram_tensor("maskv", [N], f32, kind="ExternalInput").ap()
    wqkv = nc.dram_tensor("wqkv", [6, CT, 128, 512], bf16,
                          kind="ExternalInput").ap()
    wproj = nc.dram_tensor("wproj", [2, CT, 128, 512], bf16,
                           kind="ExternalInput").ap()
    w1 = nc.dram_tensor("w1", [8, CT, 128, 512], bf16,
                        kind="ExternalInput").ap()
    w2 = nc.dram_tensor("w2", [8, 128, 4, 1024], bf16,
                        kind="ExternalInput").ap()
    sgwbqk = nc.dram_tensor("sgwbqk", [2, 2 * C], bf16,
                            kind="ExternalInput").ap()
    sgwbv = nc.dram_tensor("sgwbv", [2, C], bf16, kind="ExternalInput").ap()
    sg1 = nc.dram_tensor("sg1", [1, HID], bf16, kind="ExternalInput").ap()
    bb1g = nc.dram_tensor("bb1g", [HID], f32, kind="ExternalInput").ap()
    bproj = nc.dram_tensor("bproj", [C], f32, kind="ExternalInput").ap()
    bb2 = nc.dram_tensor("bb2", [C], f32, kind="ExternalInput").ap()
    outT = nc.dram_tensor("outT", [C, N], f32, kind="ExternalOutput").ap()

    # [C, N] DRAM viewed as two [128, 4, N] row-groups for merged DMA
    def rg(ap_, half):
        return ap_[half * 512:(half + 1) * 512, :].rearrange(
            "(a p) f -> p a f", p=128)

    with tile.TileContext(nc) as tc, \
            nc.allow_low_precision("bf16 kernel; tolerance 2e-2"):
        const_cm = tc.tile_pool(name="const", bufs=1)
        const = const_cm.__enter__()

        def vec_tiles(src_ap, n_t, name):
            t = const.tile([128, n_t], f32, name=name)
            nc.scalar.dma_start(out=t,
                                in_=src_ap.rearrange("(t p) -> p t", p=128))
            return t

        maskc = vec_tiles(maskv, NT, "maskc")
        bprojc = vec_tiles(bproj, CT, "bprojc")
        bb2c = vec_tiles(bb2, CT, "bb2c")
        bb1gc = vec_tiles(bb1g, HT, "bb1gc")
        sgqk_t = const.tile([2, 2 * C], bf16, name="sgqk")
        nc.scalar.dma_start(out=sgqk_t, in_=sgwbqk)
        sgv_t = const.tile([2, C], bf16, name="sgv")
        nc.scalar.dma_start(out=sgv_t, in_=sgwbv)
        sg1_t = const.tile([1, HID], bf16, name="sg1t")
        nc.scalar.dma_start(out=sg1_t, in_=sg1)
        onesP = const.tile([128, 128], bf16, name="onesP")
        nc.vector.memset(onesP, 1.0)
        ones64 = const.tile([1, D], bf16, name="ones64")
        nc.vector.memset(ones64, 1.0)
        epsc = const.tile([128, 1], f32, name="epsc")
        nc.vector.memset(epsc, EPS)

        def ln_chain(work, ps_sum, ps_sq, tag):
            """Post-stats chain: returns rstdB (bf16 [128,N]) and meanB."""
            meanB = work.tile([128, N], f32, tag="meanB", name=f"meanB{tag}")
            nc.vector.tensor_scalar_mul(meanB, ps_sum, 1.0 / C)
            msq = work.tile([128, N], f32, tag="msq", name=f"msq{tag}")
            nc.vector.tensor_mul(msq, meanB, meanB)
            varB = work.tile([128, N], f32, tag="varB", name=f"varB{tag}")
            nc.vector.scalar_tensor_tensor(varB, in0=ps_sq, scalar=1.0 / C,
                                           in1=msq, op0=ALU.mult,
                                           op1=ALU.subtract)
            stdB = work.tile([128, N], bf16, tag="stdB", name=f"stdB{tag}")
            nc.scalar.activation(out=stdB, in_=varB, func=AF.Sqrt, bias=epsc,
                                 scale=1.0)
            rstdB = work.tile([128, N], bf16, tag="rstdB", name=f"rstdB{tag}")
            nc.vector.reciprocal(rstdB, stdB)
            return rstdB, meanB

        for _rep in range(repeat):
            # ============ x load + weight prefetch (SP queue) ============
            px2_cm = tc.tile_pool(name="p_x2", bufs=1, side="right")
            px2 = px2_cm.__enter__()
            px_cm = tc.tile_pool(name="p_x", bufs=1, side="right")
            px = px_cm.__enter__()
            xtsB = []
            for g in range(2):
                t = px.tile([128, 4, N], bf16, tag="xts", bufs=2,
                            name=f"xts{g}")
                nc.sync.dma_start(out=t, in_=rg(xT, g))
                xtsB.append(t)

            def xslice(ct):
                return xtsB[ct // 4][:, ct % 4, :]

            pyt_cm = tc.tile_pool(name="p_yt", bufs=1, side="right")
            pyt = pyt_cm.__enter__()
            yt = [pyt.tile([128, N], bf16, tag="yt", bufs=NT, name=f"yt{j}")
                  for j in range(NT)]
            wp_cm = tc.tile_pool(name="wp_pool", bufs=1, side="right")
            wpp = wp_cm.__enter__()
            # ==================== LN1 (z-form) ======================
            pz1_cm = tc.tile_pool(name="p_z1", bufs=1, side="left")
            pz1 = pz1_cm.__enter__()
            lnsq_cm = tc.tile_pool(name="ln1_sq", bufs=1, side="left")
            lnsq = lnsq_cm.__enter__()
            ln1w_cm = tc.tile_pool(name="ln1_w", bufs=1, side="left")
            ln1w = ln1w_cm.__enter__()
            lnps_cm = tc.tile_pool(name="ln1_ps", bufs=1, space="PSUM")
            lnps = lnps_cm.__enter__()

            ps_sum = lnps.tile([128, N], f32, tag="lnsum", name="ps_sum1")
            ps_sq = lnps.tile([128, N], f32, tag="lnsq", name="ps_sq1")
            sq_tiles = []
            for ct in range(CT):
                sq = lnsq.tile([128, N], bf16, tag="sqt", bufs=CT,
                               name=f"sq1_{ct}")
                nc.gpsimd.tensor_mul(sq, xslice(ct), xslice(ct))
                sq_tiles.append(sq)
            for ch in range(2):
                cs = slice(ch * 512, (ch + 1) * 512)
                for ct in range(CT):
                    nc.tensor.matmul(ps_sum[:, cs], lhsT=onesP,
                                     rhs=xslice(ct)[:, cs],
                                     start=(ct == 0), stop=(ct == CT - 1))
            for ch in range(2):
                cs = slice(ch * 512, (ch + 1) * 512)
                for ct in range(CT):
                    nc.tensor.matmul(ps_sq[:, cs], lhsT=onesP,
                                     rhs=sq_tiles[ct][:, cs],
                                     start=(ct == 0), stop=(ct == CT - 1))
            rstd1, mean1 = ln_chain(ln1w, ps_sum, ps_sq, "1")

            z1 = []
            for ct in range(CT):
                z = pz1.tile([128, N], bf16, tag="z1", bufs=CT,
                             name=f"z1_{ct}")
                nc.vector.tensor_mul(z, xslice(ct), rstd1)
                z1.append(z)
            m1ones = pz1.tile([2, N], bf16, name="m1ones")
            nc.vector.memset(m1ones, 1.0)
            nc.vector.scalar_tensor_tensor(m1ones[0:1, :],
                                           in0=mean1[0:1, :], scalar=-1.0,
                                           in1=rstd1[0:1, :],
                                           op0=ALU.mult, op1=ALU.mult)
            lnps_cm.__exit__(None, None, None)
            ln1w_cm.__exit__(None, None, None)
            lnsq_cm.__exit__(None, None, None)

            if stop_after == "ln1":
                wp_cm.__exit__(None, None, None)
                pyt_cm.__exit__(None, None, None)
                px_cm.__exit__(None, None, None)
                px2_cm.__exit__(None, None, None)
                pz1_cm.__exit__(None, None, None)
                continue

            # ======================= V =============================
            wptiles = {}
            for fg in range(2):
                for cg in range(2):
                    wt = wpp.tile([128, 4, 512], bf16, tag="wproj", bufs=4,
                                  name=f"wp{fg}_{cg}")
                    nc.sync.dma_start(
                        out=wt,
                        in_=wproj[fg, 4 * cg:4 * cg + 4].rearrange(
                            "a p f -> p a f"))
                    wptiles[(fg, cg)] = wt
            pv_cm = tc.tile_pool(name="p_v", bufs=1, side="right")
            pv = pv_cm.__enter__()
            vkt = [pv.tile([128, H * (D + 1)], bf16, tag="vkt", bufs=NT,
                           name=f"vkt{kt}") for kt in range(NT)]
            for kt in range(NT):
                vcol = vkt[kt].rearrange("p (h u) -> p h u", u=D + 1)
                nc.gpsimd.memset(vcol[:, :, D:D + 1], 1.0)

            pqkt_cm = tc.tile_pool(name="p_qkt", bufs=1, side="right")
            pqkt = pqkt_cm.__enter__()
            qkt = [pqkt.tile([128, N], bf16, tag="qkt", bufs=16,
                             name=f"qkt{t}") for t in range(16)]

            wq_cm = tc.tile_pool(name="wq_pool", bufs=1, side="right")
            wqp = wq_cm.__enter__()
            wtiles = {}
            for ftg in (4, 5, 0, 2, 1, 3):  # v first, then q/k interleaved
                for cg in range(2):
                    wt = wqp.tile([128, 4, 512], bf16, tag="wqkv", bufs=12,
                                  name=f"wq{ftg}_{cg}")
                    nc.sync.dma_start(
                        out=wt,
                        in_=wqkv[ftg, 4 * cg:4 * cg + 4].rearrange(
                            "a p f -> p a f"))
                    wtiles[(ftg, cg)] = wt

            def wqslice(ftg, ct, fs):
                return wtiles[(ftg, ct // 4)][:, ct % 4, fs]

            vps_cm = tc.tile_pool(name="v_ps", bufs=1, space="PSUM")
            vps = vps_cm.__enter__()
            for nt in range(NT):
                ns = slice(nt * 128, (nt + 1) * 128)
                for ch in range(2):
                    ps = vps.tile([128, 512], f32, tag="vps", bufs=2,
                                  name=f"vps{nt}_{ch}")
                    for ct in range(CT):
                        nc.tensor.matmul(
                            ps, lhsT=z1[ct][:, ns],
                            rhs=wtiles[(4 + ch, ct // 4)][:, ct % 4, :],
                            start=(ct == 0), stop=False)
                    nc.tensor.matmul(
                        ps, lhsT=m1ones[:, ns],
                        rhs=sgv_t[:, ch * 512:(ch + 1) * 512],
                        start=False, stop=True)
                    vcol = vkt[nt].rearrange("p (h u) -> p h u", u=D + 1)
                    nc.vector.tensor_copy(
                        vcol[:, 8 * ch:8 * ch + 8, 0:D],
                        ps.rearrange("p (h d) -> p h d", d=D))
            # q/k production (same PSUM pool; pure PE stream)
            for t_pair in range(NT):
                for t in (t_pair, 8 + t_pair):
                    ftg = (0 if t < 8 else 2) + (t % 8) // 4
                    fs = slice(((t % 8) % 4) * 128, ((t % 8) % 4) * 128 + 128)
                    ps = vps.tile([128, N], f32, tag="qkpsw", bufs=2,
                                  name=f"qkps{t}")
                    for ch in range(2):
                        cs = slice(ch * 512, (ch + 1) * 512)
                        for ct in range(CT):
                            nc.tensor.matmul(ps[:, cs],
                                             lhsT=wqslice(ftg, ct, fs),
                                             rhs=z1[ct][:, cs],
                                             start=(ct == 0), stop=False)
                        nc.tensor.matmul(
                            ps[:, cs], lhsT=sgqk_t[:, t * 128:(t + 1) * 128],
                            rhs=m1ones[:, cs], start=False, stop=True)
                    nc.vector.tensor_copy(qkt[t], ps)
            vps_cm.__exit__(None, None, None)
            wq_cm.__exit__(None, None, None)

            # ====================== attention ==========================
            asb_cm = tc.tile_pool(name="attn_sb", bufs=1, side="right")
            asb = asb_cm.__enter__()
            aps_cm = tc.tile_pool(name="attn_ps", bufs=1, space="PSUM")
            aps = aps_cm.__enter__()

            if stop_after == "vqk":
                aps_cm.__exit__(None, None, None)
                asb_cm.__exit__(None, None, None)
                pqkt_cm.__exit__(None, None, None)
                pv_cm.__exit__(None, None, None)
                wp_cm.__exit__(None, None, None)
                pyt_cm.__exit__(None, None, None)
                px_cm.__exit__(None, None, None)
                px2_cm.__exit__(None, None, None)
                pz1_cm.__exit__(None, None, None)
                continue

            for j in range(NT):
                for h in (2 * j, 2 * j + 1):
                    hs = slice(64 * (h % 2), 64 * (h % 2) + 64)
                    ya = aps.tile([D + 1, N], f32, tag="ya", bufs=2,
                                  name=f"ya{h}")
                    pend = []
                    started = [False, False]

                    def flush_one():
                        kt_, ea_ = pend.pop(0)
                        va = vkt[kt_][:, h * (D + 1):(h + 1) * (D + 1)]
                        for ch_ in range(2):
                            cs_ = slice(ch_ * 512, (ch_ + 1) * 512)
                            nc.tensor.matmul(ya[:, cs_], lhsT=va,
                                             rhs=ea_[:, cs_],
                                             start=not started[ch_],
                                             stop=(kt_ == NT - 1))
                            started[ch_] = True

                    for kt in range(NT):
                        ks = slice(kt * 128, (kt + 1) * 128)
                        sa = aps.tile([128, N], f32, tag="sa", bufs=2,
                                      name=f"sa{h}_{kt}")
                        for ch in range(2):
                            cs = slice(ch * 512, (ch + 1) * 512)
                            nc.tensor.matmul(sa[:, cs],
                                             lhsT=qkt[8 + j][hs, ks],
                                             rhs=qkt[j][hs, cs],
                                             start=True, stop=True)
                        ea = asb.tile([128, N], bf16, tag="ea", bufs=6,
                                      name=f"ea{h}_{kt}")
                        nc.scalar.activation(out=ea, in_=sa, func=AF.Exp,
                                             bias=maskc[:, kt:kt + 1],
                                             scale=SCALE)
                        pend.append((kt, ea))
                        if len(pend) > 2:
                            flush_one()
                    while pend:
                        flush_one()
                    if stop_after not in ("attn_notail", "attn_deep",
                                          "attn_fixed"):
                        rr = asb.tile([1, N], bf16, tag="rr", bufs=3,
                                      name=f"rr{h}")
                        nc.vector.reciprocal(rr, ya[D:D + 1, :])
                        rb = asb.tile([D, N], bf16, tag="rb", bufs=3,
                                      name=f"rb{h}")
                        nc.gpsimd.partition_broadcast(rb, rr)
                        nc.vector.tensor_mul(yt[j][hs, :], ya[0:D, :], rb)

            aps_cm.__exit__(None, None, None)
            asb_cm.__exit__(None, None, None)
            pqkt_cm.__exit__(None, None, None)
            pv_cm.__exit__(None, None, None)
            pz1_cm.__exit__(None, None, None)

            if stop_after in ("attn", "attn_notail", "attn_deep",
                              "attn_fixed"):
                wp_cm.__exit__(None, None, None)
                pyt_cm.__exit__(None, None, None)
                px_cm.__exit__(None, None, None)
                px2_cm.__exit__(None, None, None)
                continue

            # ======== proj + residual + LN2 stats (interleaved) ========
            x2t = [px2.tile([128, N], bf16, tag="x2", bufs=CT,
                            name=f"x2_{o}") for o in range(CT)]
            pz2_cm = tc.tile_pool(name="p_z2", bufs=1, side="left")
            pz2 = pz2_cm.__enter__()
            ln2sq_cm = tc.tile_pool(name="ln2_sq", bufs=1, side="left")
            ln2sq = ln2sq_cm.__enter__()
            ln2w_cm = tc.tile_pool(name="ln2_w", bufs=1, side="left")
            ln2w = ln2w_cm.__enter__()
            pps_cm = tc.tile_pool(name="proj_ps", bufs=1, space="PSUM")
            pps = pps_cm.__enter__()
            lnps2_cm = tc.tile_pool(name="ln2_ps", bufs=1, space="PSUM")
            lnps2 = lnps2_cm.__enter__()

            ps_sum2 = lnps2.tile([128, N], f32, tag="lnsum", name="ps_sum2")
            ps_sq2 = lnps2.tile([128, N], f32, tag="lnsq", name="ps_sq2")
            sq2 = []

            def stats2(o):
                # lagged stats matmuls: x2/sq2 long since produced
                for ch in range(2):
                    cs = slice(ch * 512, (ch + 1) * 512)
                    nc.tensor.matmul(ps_sum2[:, cs], lhsT=onesP,
                                     rhs=x2t[o][:, cs],
                                     start=(o == 0), stop=(o == CT - 1))
                for ch in range(2):
                    cs = slice(ch * 512, (ch + 1) * 512)
                    nc.tensor.matmul(ps_sq2[:, cs], lhsT=onesP,
                                     rhs=sq2[o][:, cs],
                                     start=(o == 0), stop=(o == CT - 1))

            for o in range(CT):
                ps = pps.tile([128, N], f32, tag="pps", bufs=2,
                              name=f"pps{o}")
                fs = slice((o % 4) * 128, (o % 4 + 1) * 128)
                for ct in range(CT):
                    for ch in range(2):
                        cs = slice(ch * 512, (ch + 1) * 512)
                        nc.tensor.matmul(
                            ps[:, cs],
                            lhsT=wptiles[(o // 4, ct // 4)][:, ct % 4, fs],
                            rhs=yt[ct][:, cs],
                            start=(ct == 0), stop=(ct == CT - 1))
                nc.vector.scalar_tensor_tensor(
                    x2t[o], in0=ps, scalar=bprojc[:, o:o + 1],
                    in1=xslice(o), op0=ALU.add, op1=ALU.add)
                sq = ln2sq.tile([128, N], bf16, tag="sqt", bufs=CT,
                                name=f"sq2_{o}")
                nc.gpsimd.tensor_mul(sq, x2t[o], x2t[o])
                sq2.append(sq)
                if o >= 3:
                    stats2(o - 3)
            stats2(CT - 3)
            stats2(CT - 2)
            stats2(CT - 1)
            rstd2, mean2 = ln_chain(ln2w, ps_sum2, ps_sq2, "2")

            z2 = []
            for ct in range(CT):
                z = pz2.tile([128, N], bf16, tag="z2", bufs=CT,
                             name=f"z2_{ct}")
                nc.vector.tensor_mul(z, x2t[ct], rstd2)
                z2.append(z)
            m2row = pz2.tile([1, N], bf16, name="m2row2")
            nc.vector.scalar_tensor_tensor(m2row, in0=mean2[0:1, :],
                                           scalar=-1.0, in1=rstd2[0:1, :],
                                           op0=ALU.mult, op1=ALU.mult)
            lnps2_cm.__exit__(None, None, None)
            pps_cm.__exit__(None, None, None)
            ln2w_cm.__exit__(None, None, None)
            ln2sq_cm.__exit__(None, None, None)
            wp_cm.__exit__(None, None, None)
            pyt_cm.__exit__(None, None, None)
            px_cm.__exit__(None, None, None)

            if stop_after == "proj":
                pz2_cm.__exit__(None, None, None)
                px2_cm.__exit__(None, None, None)
                continue

            # ======================== MLP ==========================
            pht_cm = tc.tile_pool(name="p_ht", bufs=1, side="left")
            pht = pht_cm.__enter__()
            ht = [pht.tile([128, N], bf16, tag="ht", bufs=HT,
                           name=f"ht{f}") for f in range(HT)]
            osb_cm = tc.tile_pool(name="out_sb", bufs=1, side="left")
            osb = osb_cm.__enter__()
            mw_cm = tc.tile_pool(name="mw_pool", bufs=1, side="left")
            mwp = mw_cm.__enter__()
            mps_cm = tc.tile_pool(name="mlp_ps", bufs=1, space="PSUM")
            mps = mps_cm.__enter__()

            w1tiles = {}

            def w1_load(fg):
                for cg in range(2):
                    wt = mwp.tile([128, 4, 512], bf16, tag="w1",
                                  bufs=5, name=f"w1_{fg}_{cg}")
                    nc.sync.dma_start(
                        out=wt,
                        in_=w1[fg, 4 * cg:4 * cg + 4].rearrange(
                            "a p f -> p a f"))
                    w1tiles[(fg, cg)] = wt

            for fg in range(2):
                w1_load(fg)
            # w2 all-resident: 8 tiles [128, 4, 1024] (SP queue)
            w2tiles = []
            for fg in range(8):
                wt = mwp.tile([128, 4, N], bf16, tag="w2", bufs=8,
                              name=f"w2_{fg}")
                nc.sync.dma_start(out=wt, in_=w2[fg])
                w2tiles.append(wt)

            for f in range(HT):
                fg, fi = f // 4, f % 4
                if fi == 0 and fg >= 2:
                    w1_load(fg)
                ps = mps.tile([128, N], f32, tag="fc1ps", bufs=2,
                              name=f"m1ps{f}")
                fs = slice(fi * 128, (fi + 1) * 128)
                for ch in range(2):
                    cs = slice(ch * 512, (ch + 1) * 512)
                    for ct in range(CT):
                        nc.tensor.matmul(
                            ps[:, cs],
                            lhsT=w1tiles[(fg, ct // 4)][:, ct % 4, fs],
                            rhs=z2[ct][:, cs],
                            start=(ct == 0), stop=False)
                    nc.tensor.matmul(
                        ps[:, cs], lhsT=sg1_t[:, f * 128:(f + 1) * 128],
                        rhs=m2row[:, cs], start=False, stop=True)
                nc.scalar.activation(out=ht[f], in_=ps, func=AF.Gelu,
                                     bias=bb1gc[:, f:f + 1], scale=1.0)

            # fc2: o-major full-depth PSUM accumulation
            for o in range(CT):
                ps = mps.tile([128, N], f32, tag="fc2ps", bufs=2,
                              name=f"m2ps{o}")
                os_ = slice((o % 4) * 128, (o % 4 + 1) * 128 + 0)
                for fl in range(HT):
                    for ch in range(2):
                        cs = slice(ch * 512, (ch + 1) * 512)
                        nc.tensor.matmul(
                            ps[:, cs],
                            lhsT=w2tiles[fl // 4][:, fl % 4,
                                                  o * 128:(o + 1) * 128],
                            rhs=ht[fl][:, cs],
                            start=(fl == 0), stop=(fl == HT - 1))
                ot = osb.tile([128, N], f32, tag="ot", bufs=2,
                              name=f"ot{o}")
                nc.vector.scalar_tensor_tensor(
                    ot, in0=ps, scalar=bb2c[:, o:o + 1], in1=x2t[o],
                    op0=ALU.add, op1=ALU.add)
                nc.gpsimd.dma_start(out=outT[o * 128:(o + 1) * 128, :],
                                    in_=ot)

            mps_cm.__exit__(None, None, None)
            mw_cm.__exit__(None, None, None)
            osb_cm.__exit__(None, None, None)
            pht_cm.__exit__(None, None, None)
            pz2_cm.__exit__(None, None, None)
            px2_cm.__exit__(None, None, None)

        const_cm.__exit__(None, None, None)

    nc.compile()
    return nc


_NC_CACHE = {}


def _get_program():
    if "nc" not in _NC_CACHE:
        _NC_CACHE["nc"] = build_program()
    return _NC_CACHE["nc"]


def _til(WT, n_fg):
    # WT: [K, M] (contraction-major); -> [n_fg, K//128, 128, 512]
    K, M = WT.shape
    return np.ascontiguousarray(
        WT.reshape(K // 128, 128, n_fg, 512).transpose(2, 0, 1, 3))


def _prep_inputs(x, length, g1, b1, Wqkv, Wproj, bproj, g2, b2, W1, bb1, W2,
                 bb2):
    """Host-side prep: LN-folded weights in bf16 + per-core in_maps."""
    x = np.asarray(x, np.float32)
    length = np.asarray(length)
    g1 = np.asarray(g1, np.float32); b1 = np.asarray(b1, np.float32)
    g2 = np.asarray(g2, np.float32); b2 = np.asarray(b2, np.float32)
    bproj = np.asarray(bproj, np.float32)
    bb1 = np.asarray(bb1, np.float32); bb2 = np.asarray(bb2, np.float32)
    Wqkv = np.asarray(Wqkv, np.float32); Wproj = np.asarray(Wproj, np.float32)
    W1 = np.asarray(W1, np.float32); W2 = np.asarray(W2, np.float32)

    Wg1 = Wqkv * g1[None, :]
    sg_qkv = Wg1.sum(1)
    wb_qkv = Wqkv @ b1
    Wg2 = W1 * g2[None, :]

    wts = {
        "wqkv": _til(np.ascontiguousarray(Wg1.T), 6).astype(BF),
        "wproj": _til(np.ascontiguousarray(Wproj.T), 2).astype(BF),
        "w1": _til(np.ascontiguousarray(Wg2.T), 8).astype(BF),
        "w2": np.ascontiguousarray(
            W2.T.reshape(8, 4, 128, C).transpose(0, 2, 1, 3)).astype(BF),
        "sgwbqk": np.stack([sg_qkv[:2 * C], wb_qkv[:2 * C]]).astype(BF),
        "sgwbv": np.stack([sg_qkv[2 * C:], wb_qkv[2 * C:]]).astype(BF),
        "sg1": Wg2.sum(1)[None, :].astype(BF),
        "bb1g": (bb1 + W1 @ b2).astype(np.float32),
        "bproj": bproj, "bb2": bb2,
    }
    xTb = np.ascontiguousarray(x.transpose(0, 2, 1)).astype(BF)  # [B, C, N]
    mask = (np.arange(N)[None, :] >= length[:, None]).astype(
        np.float32) * MASK_NEG  # [B, N]
    return [dict(wts, xT=xTb[b], maskv=np.ascontiguousarray(mask[b]))
            for b in range(B)]


def kernel(x, length, g1, b1, Wqkv, Wproj, bproj, g2, b2, W1, bb1, W2, bb2):
    in_maps = _prep_inputs(x, length, g1, b1, Wqkv, Wproj, bproj, g2, b2,
                           W1, bb1, W2, bb2)
    nc = _get_program()
    res = run_bass_kernel_spmd(nc, in_maps, core_ids=list(range(NCORES)))
    out = np.stack([res.results[b]["outT"] for b in range(B)], axis=0)
    return np.ascontiguousarray(out.transpose(0, 2, 1))


# revision 22
# speedup vs baseline: 1.0957x; 1.0957x over previous
"""Trainium2 Bass kernel for a dense transformer block (B=8, N=1024, C=1024,
H=16, D=64, HID=4096) with padding-masked attention.

Sharding: data-parallel over batch - one batch element per NeuronCore.

v2 design (vs v1 baseline):
- All matmul operands bf16 (weights host-converted; activations converted on
  the fly by the producing engine).  Halves DMA + SBUF, keeps 1 cyc/row PE.
- LayerNorm normalize folded into the following matmul:  LN(x)@Wg =
  (x*rstd)@(W*g) + rank-1 correction (-mu*rstd outer sum(W*g)) + W@b.  The
  rank-1 + bias terms ride the PSUM accumulation as one extra K<=2 matmul.
  Kills the per-tile sub/scale/bias elementwise chain of v1.
- Softmax denominator reciprocal broadcast via gpsimd partition_broadcast
  (SBUF only) instead of a DRAM DMA round-trip on the ACT queue.
- Scores processed in [128,512] PSUM chunks; exp on ACT; QKV q/k production
  matmuls interleaved into the attention stream so PE never idles while ACT
  exps run.
- proj + residual + LN2 stats interleaved; fc2 accumulates o-major in PSUM
  (no SBUF accumulation adds); x / x2 stay in SBUF (no DRAM spill).
- Engine discipline: SP queue = weight/x prefetch only; Pool = squares +
  partition broadcasts; DVE = everything touching PSUM; ACT = exp/gelu/sqrt.
"""

import os
import sys

for _p in ("/opt/trn_rl_repo",):
    if _p not in sys.path:
        sys.path.insert(0, _p)
os.environ.setdefault("MYCRO_LOCAL_CACHE", "1")

import numpy as np  # noqa: E402
import ml_dtypes  # noqa: E402

import concourse.bacc as bacc  # noqa: E402
import concourse.tile as tile  # noqa: E402
from concourse import mybir  # noqa: E402
from concourse.bass_utils import run_bass_kernel_spmd  # noqa: E402

f32 = mybir.dt.float32
bf16 = mybir.dt.bfloat16
AF = mybir.ActivationFunctionType
ALU = mybir.AluOpType
BF = ml_dtypes.bfloat16

B, N, C = 8, 1024, 1024
H, D = 16, 64
HID = 4 * C
CT = C // 128          # 8 c-tiles
NT = N // 128          # 8 n/k-tiles
HT = HID // 128        # 32 hid-tiles
SCALE = D ** -0.5
EPS = 1e-5
MASK_NEG = -10000.0

NCORES = 8


def build_program(repeat=1, stop_after=None):
    nc = bacc.Bacc("TRN2", target_bir_lowering=False, debug=False)

    xT = nc.dram_tensor("xT", [C, N], bf16, kind="ExternalInput").ap()
    maskv = nc.dram_tensor("maskv", [N], f32, kind="ExternalInput").ap()
    wqkv = nc.dram_tensor("wqkv", [6, CT, 128, 512], bf16,
                          kind="ExternalInput").ap()
    wproj = nc.dram_tensor("wproj", [2, CT, 128, 512], bf16,
                           kind="ExternalInput").ap()
    w1 = nc.dram_tensor("w1", [8, CT, 128, 512], bf16,
                        kind="ExternalInput").ap()
    w2 = nc.dram_tensor("w2", [8, 128, 4, 1024], bf16,
                        kind="ExternalInput").ap()
    sgwbqk = nc.dram_tensor("sgwbqk", [2, 2 * C], bf16,
                            kind="ExternalInput").ap()
    sgwbv = nc.dram_tensor("sgwbv", [2, C], bf16, kind="ExternalInput").ap()
    sg1 = nc.dram_tensor("sg1", [1, HID], bf16, kind="ExternalInput").ap()
    bb1g = nc.dram_tensor("bb1g", [HID], f32, kind="ExternalInput").ap()
    bproj = nc.dram_tensor("bproj", [C], f32, kind="ExternalInput").ap()
    bb2 = nc.dram_tensor("bb2", [C], f32, kind="ExternalInput").ap()
    outT = nc.dram_tensor("outT", [C, N], f32, kind="ExternalOutput").ap()

    # [C, N] DRAM viewed as two [128, 4, N] row-groups for merged DMA
    def rg(ap_, half):
        return ap_[half * 512:(half + 1) * 512, :].rearrange(
            "(a p) f -> p a f", p=128)

    with tile.TileContext(nc) as tc, \
            nc.allow_low_precision("bf16 kernel; tolerance 2e-2"):
        const_cm = tc.tile_pool(name="const", bufs=1)
        const = const_cm.__enter__()

        def vec_tiles(src_ap, n_t, name):
            t = const.tile([128, n_t], f32, name=name)
            nc.scalar.dma_start(out=t,
                                in_=src_ap.rearrange("(t p) -> p t", p=128))
            return t

        maskc = vec_tiles(maskv, NT, "maskc")
        bprojc = vec_tiles(bproj, CT, "bprojc")
        bb2c = vec_tiles(bb2, CT, "bb2c")
        bb1gc = vec_tiles(bb1g, HT, "bb1gc")
        sgqk_t = const.tile([2, 2 * C], bf16, name="sgqk")
        nc.scalar.dma_start(out=sgqk_t, in_=sgwbqk)
        sgv_t = const.tile([2, C], bf16, name="sgv")
        nc.scalar.dma_start(out=sgv_t, in_=sgwbv)
        sg1_t = const.tile([1, HID], bf16, name="sg1t")
        nc.scalar.dma_start(out=sg1_t, in_=sg1)
        onesP = const.tile([128, 128], bf16, name="onesP")
        nc.vector.memset(onesP, 1.0)
        ones64 = const.tile([1, D], bf16, name="ones64")
        nc.vector.memset(ones64, 1.0)
        epsc = const.tile([128, 1], f32, name="epsc")
        nc.vector.memset(epsc, EPS)

        def ln_chain(work, ps_sum, ps_sq, tag):
            """Post-stats chain: returns rstdB (bf16 [128,N]) and meanB."""
            meanB = work.tile([128, N], f32, tag="meanB", name=f"meanB{tag}")
            nc.vector.tensor_scalar_mul(meanB, ps_sum, 1.0 / C)
            msq = work.tile([128, N], f32, tag="msq", name=f"msq{tag}")
            nc.vector.tensor_mul(msq, meanB, meanB)
            varB = work.tile([128, N], f32, tag="varB", name=f"varB{tag}")
            nc.vector.scalar_tensor_tensor(varB, in0=ps_sq, scalar=1.0 / C,
                                           in1=msq, op0=ALU.mult,
                                           op1=ALU.subtract)
            stdB = work.tile([128, N], bf16, tag="stdB", name=f"stdB{tag}")
            nc.scalar.activation(out=stdB, in_=varB, func=AF.Sqrt, bias=epsc,
                                 scale=1.0)
            rstdB = work.tile([128, N], bf16, tag="rstdB", name=f"rstdB{tag}")
            nc.vector.reciprocal(rstdB, stdB)
            return rstdB, meanB

        for _rep in range(repeat):
            # ============ x load + weight prefetch (SP queue) ============
            px2_cm = tc.tile_pool(name="p_x2", bufs=1, side="right")
            px2 = px2_cm.__enter__()
            px_cm = tc.tile_pool(name="p_x", bufs=1, side="right")
            px = px_cm.__enter__()
            xtsB = []
            for g in range(2):
                t = px.tile([128, 4, N], bf16, tag="xts", bufs=2,
                            name=f"xts{g}")
                nc.sync.dma_start(out=t, in_=rg(xT, g))
                xtsB.append(t)

            def xslice(ct):
                return xtsB[ct // 4][:, ct % 4, :]

            pyt_cm = tc.tile_pool(name="p_yt", bufs=1, side="right")
            pyt = pyt_cm.__enter__()
            yt = [pyt.tile([128, N], bf16, tag="yt", bufs=NT, name=f"yt{j}")
                  for j in range(NT)]
            wp_cm = tc.tile_pool(name="wp_pool", bufs=1, side="right")
            wpp = wp_cm.__enter__()
            # ==================== LN1 (z-form) ======================
            pz1_cm = tc.tile_pool(name="p_z1", bufs=1, side="left")
            pz1 = pz1_cm.__enter__()
            lnsq_cm = tc.tile_pool(name="ln1_sq", bufs=1, side="left")
            lnsq = lnsq_cm.__enter__()
            ln1w_cm = tc.tile_pool(name="ln1_w", bufs=1, side="left")
            ln1w = ln1w_cm.__enter__()
            lnps_cm = tc.tile_pool(name="ln1_ps", bufs=1, space="PSUM")
            lnps = lnps_cm.__enter__()

            ps_sum = lnps.tile([128, N], f32, tag="lnsum", name="ps_sum1")
            ps_sq = lnps.tile([128, N], f32, tag="lnsq", name="ps_sq1")
            sq_tiles = []
            for ct in range(CT):
                sq = lnsq.tile([128, N], bf16, tag="sqt", bufs=CT,
                               name=f"sq1_{ct}")
                nc.gpsimd.tensor_mul(sq, xslice(ct), xslice(ct))
                sq_tiles.append(sq)
            for ch in range(2):
                cs = slice(ch * 512, (ch + 1) * 512)
                for ct in range(CT):
                    nc.tensor.matmul(ps_sum[:, cs], lhsT=onesP,
                                     rhs=xslice(ct)[:, cs],
                                     start=(ct == 0), stop=(ct == CT - 1))
            for ch in range(2):
                cs = slice(ch * 512, (ch + 1) * 512)
                for ct in range(CT):
                    nc.tensor.matmul(ps_sq[:, cs], lhsT=onesP,
                                     rhs=sq_tiles[ct][:, cs],
                                     start=(ct == 0), stop=(ct == CT - 1))
            rstd1, mean1 = ln_chain(ln1w, ps_sum, ps_sq, "1")

            z1 = []
            for ct in range(CT):
                z = pz1.tile([128, N], bf16, tag="z1", bufs=CT,
                             name=f"z1_{ct}")
                nc.vector.tensor_mul(z, xslice(ct), rstd1)
                z1.append(z)
            m1ones = pz1.tile([2, N], bf16, name="m1ones")
            nc.vector.memset(m1ones, 1.0)
            nc.vector.scalar_tensor_tensor(m1ones[0:1, :],
                                           in0=mean1[0:1, :], scalar=-1.0,
                                           in1=rstd1[0:1, :],
                                           op0=ALU.mult, op1=ALU.mult)
            lnps_cm.__exit__(None, None, None)
            ln1w_cm.__exit__(None, None, None)
            lnsq_cm.__exit__(None, None, None)

            if stop_after == "ln1":
                wp_cm.__exit__(None, None, None)
                pyt_cm.__exit__(None, None, None)
                px_cm.__exit__(None, None, None)
                px2_cm.__exit__(None, None, None)
                pz1_cm.__exit__(None, None, None)
                continue

            # ======================= V =============================
            wptiles = {}
            for fg in range(2):
                for cg in range(2):
                    wt = wpp.tile([128, 4, 512], bf16, tag="wproj", bufs=4,
                                  name=f"wp{fg}_{cg}")
                    nc.sync.dma_start(
                        out=wt,
                        in_=wproj[fg, 4 * cg:4 * cg + 4].rearrange(
                            "a p f -> p a f"))
                    wptiles[(fg, cg)] = wt
            pv_cm = tc.tile_pool(name="p_v", bufs=1, side="right")
            pv = pv_cm.__enter__()
            vkt = [pv.tile([128, H * (D + 1)], bf16, tag="vkt", bufs=NT,
                           name=f"vkt{kt}") for kt in range(NT)]
            for kt in range(NT):
                vcol = vkt[kt].rearrange("p (h u) -> p h u", u=D + 1)
                nc.gpsimd.memset(vcol[:, :, D:D + 1], 1.0)

            pqkt_cm = tc.tile_pool(name="p_qkt", bufs=1, side="right")
            pqkt = pqkt_cm.__enter__()
            qkt = [pqkt.tile([128, N], bf16, tag="qkt", bufs=16,
                             name=f"qkt{t}") for t in range(16)]

            wq_cm = tc.tile_pool(name="wq_pool", bufs=1, side="right")
            wqp = wq_cm.__enter__()
            wtiles = {}
            for ftg in (4, 5, 0, 2, 1, 3):  # v first, then q/k interleaved
                for cg in range(2):
                    wt = wqp.tile([128, 4, 512], bf16, tag="wqkv", bufs=12,
                                  name=f"wq{ftg}_{cg}")
                    nc.sync.dma_start(
                        out=wt,
                        in_=wqkv[ftg, 4 * cg:4 * cg + 4].rearrange(
                            "a p f -> p a f"))
                    wtiles[(ftg, cg)] = wt

            def wqslice(ftg, ct, fs):
                return wtiles[(ftg, ct // 4)][:, ct % 4, fs]

            vps_cm = tc.tile_pool(name="v_ps", bufs=1, space="PSUM")
            vps = vps_cm.__enter__()
            for nt in range(NT):
                ns = slice(nt * 128, (nt + 1) * 128)
                for ch in range(2):
                    ps = vps.tile([128, 512], f32, tag="vps", bufs=2,
                                  name=f"vps{nt}_{ch}")
                    for ct in range(CT):
                        nc.tensor.matmul(
                            ps, lhsT=z1[ct][:, ns],
                            rhs=wtiles[(4 + ch, ct // 4)][:, ct % 4, :],
                            start=(ct == 0), stop=False)
                    nc.tensor.matmul(
                        ps, lhsT=m1ones[:, ns],
                        rhs=sgv_t[:, ch * 512:(ch + 1) * 512],
                        start=False, stop=True)
                    vcol = vkt[nt].rearrange("p (h u) -> p h u", u=D + 1)
                    nc.vector.tensor_copy(
                        vcol[:, 8 * ch:8 * ch + 8, 0:D],
                        ps.rearrange("p (h d) -> p h d", d=D))
            # q/k production (same PSUM pool; pure PE stream)
            for t_pair in range(NT):
                for t in (t_pair, 8 + t_pair):
                    ftg = (0 if t < 8 else 2) + (t % 8) // 4
                    fs = slice(((t % 8) % 4) * 128, ((t % 8) % 4) * 128 + 128)
                    ps = vps.tile([128, N], f32, tag="qkpsw", bufs=2,
                                  name=f"qkps{t}")
                    for ch in range(2):
                        cs = slice(ch * 512, (ch + 1) * 512)
                        for ct in range(CT):
                            nc.tensor.matmul(ps[:, cs],
                                             lhsT=wqslice(ftg, ct, fs),
                                             rhs=z1[ct][:, cs],
                                             start=(ct == 0), stop=False)
                        nc.tensor.matmul(
                            ps[:, cs], lhsT=sgqk_t[:, t * 128:(t + 1) * 128],
                            rhs=m1ones[:, cs], start=False, stop=True)
                    nc.vector.tensor_copy(qkt[t], ps)
            vps_cm.__exit__(None, None, None)
            wq_cm.__exit__(None, None, None)

            # ====================== attention ==========================
            asb_cm = tc.tile_pool(name="attn_sb", bufs=1, side="right")
            asb = asb_cm.__enter__()
            aps_cm = tc.tile_pool(name="attn_ps", bufs=1, space="PSUM")
            aps = aps_cm.__enter__()

            if stop_after == "vqk":
                aps_cm.__exit__(None, None, None)
                asb_cm.__exit__(None, None, None)
                pqkt_cm.__exit__(None, None, None)
                pv_cm.__exit__(None, None, None)
                wp_cm.__exit__(None, None, None)
                pyt_cm.__exit__(None, None, None)
                px_cm.__exit__(None, None, None)
                px2_cm.__exit__(None, None, None)
                pz1_cm.__exit__(None, None, None)
                continue

            for j in range(NT):
                for h in (2 * j, 2 * j + 1):
                    hs = slice(64 * (h % 2), 64 * (h % 2) + 64)
                    ya = aps.tile([D + 1, N], f32, tag="ya", bufs=2,
                                  name=f"ya{h}")
                    pend = []
                    started = [False, False]

                    def flush_one():
                        kt_, ea_ = pend.pop(0)
                        va = vkt[kt_][:, h * (D + 1):(h + 1) * (D + 1)]
                        for ch_ in range(2):
                            cs_ = slice(ch_ * 512, (ch_ + 1) * 512)
                            nc.tensor.matmul(ya[:, cs_], lhsT=va,
                                             rhs=ea_[:, cs_],
                                             start=not started[ch_],
                                             stop=(kt_ == NT - 1))
                            started[ch_] = True

                    for kt in range(NT):
                        ks = slice(kt * 128, (kt + 1) * 128)
                        sa = aps.tile([128, N], f32, tag="sa", bufs=2,
                                      name=f"sa{h}_{kt}")
                        for ch in range(2):
                            cs = slice(ch * 512, (ch + 1) * 512)
                            nc.tensor.matmul(sa[:, cs],
                                             lhsT=qkt[8 + j][hs, ks],
                                             rhs=qkt[j][hs, cs],
                                             start=True, stop=True)
                        ea = asb.tile([128, N], bf16, tag="ea", bufs=6,
                                      name=f"ea{h}_{kt}")
                        nc.scalar.activation(out=ea, in_=sa, func=AF.Exp,
                                             bias=maskc[:, kt:kt + 1],
                                             scale=SCALE)
                        pend.append((kt, ea))
                        if len(pend) > 2:
                            flush_one()
                    while pend:
                        flush_one()
                    if stop_after not in ("attn_notail", "attn_deep",
                                          "attn_fixed"):
                        rr = asb.tile([1, N], bf16, tag="rr", bufs=3,
                                      name=f"rr{h}")
                        nc.vector.reciprocal(rr, ya[D:D + 1, :])
                        rb = asb.tile([D, N], bf16, tag="rb", bufs=3,
                                      name=f"rb{h}")
                        nc.gpsimd.partition_broadcast(rb, rr)
                        nc.vector.tensor_mul(yt[j][hs, :], ya[0:D, :], rb)

            aps_cm.__exit__(None, None, None)
            asb_cm.__exit__(None, None, None)
            pqkt_cm.__exit__(None, None, None)
            pv_cm.__exit__(None, None, None)
            pz1_cm.__exit__(None, None, None)

            if stop_after in ("attn", "attn_notail", "attn_deep",
                              "attn_fixed"):
                wp_cm.__exit__(None, None, None)
                pyt_cm.__exit__(None, None, None)
                px_cm.__exit__(None, None, None)
                px2_cm.__exit__(None, None, None)
                continue

            # ======== proj + residual + LN2 stats (interleaved) ========
            x2t = [px2.tile([128, N], bf16, tag="x2", bufs=CT,
                            name=f"x2_{o}") for o in range(CT)]
            pz2_cm = tc.tile_pool(name="p_z2", bufs=1, side="left")
            pz2 = pz2_cm.__enter__()
            ln2sq_cm = tc.tile_pool(name="ln2_sq", bufs=1, side="left")
            ln2sq = ln2sq_cm.__enter__()
            ln2w_cm = tc.tile_pool(name="ln2_w", bufs=1, side="left")
            ln2w = ln2w_cm.__enter__()
            pps_cm = tc.tile_pool(name="proj_ps", bufs=1, space="PSUM")
            pps = pps_cm.__enter__()
            lnps2_cm = tc.tile_pool(name="ln2_ps", bufs=1, space="PSUM")
            lnps2 = lnps2_cm.__enter__()

            ps_sum2 = lnps2.tile([128, N], f32, tag="lnsum", name="ps_sum2")
            ps_sq2 = lnps2.tile([128, N], f32, tag="lnsq", name="ps_sq2")
            sq2 = []

            def stats2(o):
                # lagged stats matmuls: x2/sq2 long since produced
                for ch in range(2):
                    cs = slice(ch * 512, (ch + 1) * 512)
                    nc.tensor.matmul(ps_sum2[:, cs], lhsT=onesP,
                                     rhs=x2t[o][:, cs],
                                     start=(o == 0), stop=(o == CT - 1))
                for ch in range(2):
                    cs = slice(ch * 512, (ch + 1) * 512)
                    nc.tensor.matmul(ps_sq2[:, cs], lhsT=onesP,
                                     rhs=sq2[o][:, cs],
                                     start=(o == 0), stop=(o == CT - 1))

            for o in range(CT):
                ps = pps.tile([128, N], f32, tag="pps", bufs=2,
                              name=f"pps{o}")
                fs = slice((o % 4) * 128, (o % 4 + 1) * 128)
                for ct in range(CT):
                    for ch in range(2):
                        cs = slice(ch * 512, (ch + 1) * 512)
                        nc.tensor.matmul(
                            ps[:, cs],
                            lhsT=wptiles[(o // 4, ct // 4)][:, ct % 4, fs],
                            rhs=yt[ct][:, cs],
                            start=(ct == 0), stop=(ct == CT - 1))
                nc.vector.scalar_tensor_tensor(
                    x2t[o], in0=ps, scalar=bprojc[:, o:o + 1],
                    in1=xslice(o), op0=ALU.add, op1=ALU.add)
                sq = ln2sq.tile([128, N], bf16, tag="sqt", bufs=CT,
                                name=f"sq2_{o}")
                nc.gpsimd.tensor_mul(sq, x2t[o], x2t[o])
                sq2.append(sq)
                if o >= 2:
                    stats2(o - 2)
            stats2(CT - 2)
            stats2(CT - 1)
            rstd2, mean2 = ln_chain(ln2w, ps_sum2, ps_sq2, "2")

            z2 = []
            for ct in range(CT):
                z = pz2.tile([128, N], bf16, tag="z2", bufs=CT,
                             name=f"z2_{ct}")
                nc.vector.tensor_mul(z, x2t[ct], rstd2)
                z2.append(z)
            m2row = pz2.tile([1, N], bf16, name="m2row2")
            nc.vector.scalar_tensor_tensor(m2row, in0=mean2[0:1, :],
                                           scalar=-1.0, in1=rstd2[0:1, :],
                                           op0=ALU.mult, op1=ALU.mult)
            lnps2_cm.__exit__(None, None, None)
            pps_cm.__exit__(None, None, None)
            ln2w_cm.__exit__(None, None, None)
            ln2sq_cm.__exit__(None, None, None)
            wp_cm.__exit__(None, None, None)
            pyt_cm.__exit__(None, None, None)
            px_cm.__exit__(None, None, None)

            if stop_after == "proj":
                pz2_cm.__exit__(None, None, None)
                px2_cm.__exit__(None, None, None)
                continue

            # ======================== MLP ==========================
            pht_cm = tc.tile_pool(name="p_ht", bufs=1, side="left")
            pht = pht_cm.__enter__()
            ht = [pht.tile([128, N], bf16, tag="ht", bufs=HT,
                           name=f"ht{f}") for f in range(HT)]
            osb_cm = tc.tile_pool(name="out_sb", bufs=1, side="left")
            osb = osb_cm.__enter__()
            mw_cm = tc.tile_pool(name="mw_pool", bufs=1, side="left")
            mwp = mw_cm.__enter__()
            mps_cm = tc.tile_pool(name="mlp_ps", bufs=1, space="PSUM")
            mps = mps_cm.__enter__()

            w1tiles = {}

            def w1_load(fg):
                for cg in range(2):
                    wt = mwp.tile([128, 4, 512], bf16, tag="w1",
                                  bufs=5, name=f"w1_{fg}_{cg}")
                    nc.sync.dma_start(
                        out=wt,
                        in_=w1[fg, 4 * cg:4 * cg + 4].rearrange(
                            "a p f -> p a f"))
                    w1tiles[(fg, cg)] = wt

            for fg in range(2):
                w1_load(fg)
            # w2 all-resident: 8 tiles [128, 4, 1024] (SP queue)
            w2tiles = []
            for fg in range(8):
                wt = mwp.tile([128, 4, N], bf16, tag="w2", bufs=8,
                              name=f"w2_{fg}")
                nc.sync.dma_start(out=wt, in_=w2[fg])
                w2tiles.append(wt)

            for f in range(HT):
                fg, fi = f // 4, f % 4
                if fi == 0 and fg >= 2:
                    w1_load(fg)
                ps = mps.tile([128, N], f32, tag="fc1ps", bufs=2,
                              name=f"m1ps{f}")
                fs = slice(fi * 128, (fi + 1) * 128)
                for ch in range(2):
                    cs = slice(ch * 512, (ch + 1) * 512)
                    for ct in range(CT):
                        nc.tensor.matmul(
                            ps[:, cs],
                            lhsT=w1tiles[(fg, ct // 4)][:, ct % 4, fs],
                            rhs=z2[ct][:, cs],
                            start=(ct == 0), stop=False)
                    nc.tensor.matmul(
                        ps[:, cs], lhsT=sg1_t[:, f * 128:(f + 1) * 128],
                        rhs=m2row[:, cs], start=False, stop=True)
                nc.scalar.activation(out=ht[f], in_=ps, func=AF.Gelu,
                                     bias=bb1gc[:, f:f + 1], scale=1.0)

            # fc2: o-major full-depth PSUM accumulation
            for o in range(CT):
                ps = mps.tile([128, N], f32, tag="fc2ps", bufs=2,
                              name=f"m2ps{o}")
                os_ = slice((o % 4) * 128, (o % 4 + 1) * 128 + 0)
                for fl in range(HT):
                    for ch in range(2):
                        cs = slice(ch * 512, (ch + 1) * 512)
                        nc.tensor.matmul(
                            ps[:, cs],
                            lhsT=w2tiles[fl // 4][:, fl % 4,
                                                  o * 128:(o + 1) * 128],
                            rhs=ht[fl][:, cs],
                            start=(fl == 0), stop=(fl == HT - 1))
                ot = osb.tile([128, N], f32, tag="ot", bufs=2,
                              name=f"ot{o}")
                nc.vector.scalar_tensor_tensor(
                    ot, in0=ps, scalar=bb2c[:, o:o + 1], in1=x2t[o],
                    op0=ALU.add, op1=ALU.add)
                nc.scalar.dma_start(out=outT[o * 128:(o + 1) * 128, :],
                                    in_=ot)

            mps_cm.__exit__(None, None, None)
            mw_cm.__exit__(None, None, None)
            osb_cm.__exit__(None, None, None)
            pht_cm.__exit__(None, None, None)
            pz2_cm.__exit__(None, None, None)
            px2_cm.__exit__(None, None, None)

        const_cm.__exit__(None, None, None)

    nc.compile()
    return nc


_NC_CACHE = {}


def _get_program():
    if "nc" not in _NC_CACHE:
        _NC_CACHE["nc"] = build_program()
    return _NC_CACHE["nc"]


def _til(WT, n_fg):
    # WT: [K, M] (contraction-major); -> [n_fg, K//128, 128, 512]
    K, M = WT.shape
    return np.ascontiguousarray(
        WT.reshape(K // 128, 128, n_fg, 512).transpose(2, 0, 1, 3))


def _prep_inputs(x, length, g1, b1, Wqkv, Wproj, bproj, g2, b2, W1, bb1, W2,
                 bb2):
    """Host-side prep: LN-folded weights in bf16 + per-core in_maps."""
    x = np.asarray(x, np.float32)
    length = np.asarray(length)
    g1 = np.asarray(g1, np.float32); b1 = np.asarray(b1, np.float32)
    g2 = np.asarray(g2, np.float32); b2 = np.asarray(b2, np.float32)
    bproj = np.asarray(bproj, np.float32)
    bb1 = np.asarray(bb1, np.float32); bb2 = np.asarray(bb2, np.float32)
    Wqkv = np.asarray(Wqkv, np.float32); Wproj = np.asarray(Wproj, np.float32)
    W1 = np.asarray(W1, np.float32); W2 = np.asarray(W2, np.float32)

    Wg1 = Wqkv * g1[None, :]
    sg_qkv = Wg1.sum(1)
    wb_qkv = Wqkv @ b1
    Wg2 = W1 * g2[None, :]

    wts = {
        "wqkv": _til(np.ascontiguousarray(Wg1.T), 6).astype(BF),
        "wproj": _til(np.ascontiguousarray(Wproj.T), 2).astype(BF),
        "w1": _til(np.ascontiguousarray(Wg2.T), 8).astype(BF),
        "w2": np.ascontiguousarray(
            W2.T.reshape(8, 4, 128, C).transpose(0, 2, 1, 3)).astype(BF),
        "sgwbqk": np.stack([sg_qkv[:2 * C], wb_qkv[:2 * C]]).astype(BF),
        "sgwbv": np.stack([sg_qkv[2 * C:], wb_qkv[2 * C:]]).astype(BF),
        "sg1": Wg2.sum(1)[None, :].astype(BF),
        "bb1g": (bb1 + W1 @ b2).astype(np.float32),
        "bproj": bproj, "bb2": bb2,
    }
    xTb = np.ascontiguousarray(x.transpose(0, 2, 1)).astype(BF)  # [B, C, N]
    mask = (np.arange(N)[None, :] >= length[:, None]).astype(
        np.float32) * MASK_NEG  # [B, N]
    return [dict(wts, xT=xTb[b], maskv=np.ascontiguousarray(mask[b]))
            for b in range(B)]


def kernel(x, length, g1, b1, Wqkv, Wproj, bproj, g2, b2, W1, bb1, W2, bb2):
    in_maps = _prep_inputs(x, length, g1, b1, Wqkv, Wproj, bproj, g2, b2,
                           W1, bb1, W2, bb2)
    nc = _get_program()
    res = run_bass_kernel_spmd(nc, in_maps, core_ids=list(range(NCORES)))
    out = np.stack([res.results[b]["outT"] for b in range(B)], axis=0)
    return np.ascontiguousarray(out.transpose(0, 2, 1))
